# revision 2
# baseline (speedup 1.0000x reference)
"""Trainium2 Bass kernel for EnhancedMultiHeadAttention (LoRA MHA + residual + LayerNorm).

Contract: kernel(**inputs) takes the FULL unsharded inputs (as produced by
setup_inputs()) and returns the full outputs (normed, attn.mean(axis=1)).

Sharding: 8 cores = 4 batches x 2 query-halves. Each core computes K/V for the
whole sequence of its batch (duplicated across the pair) and attention +
output-projection + LayerNorm for its 512 query rows.

Key optimizations over the first working version (231us -> 218us):
  - K bias dropped entirely: softmax over k is invariant to the per-q constant
    q.k_b adds to scores, so neither output depends on it.
  - Interleaved emission: engine queues execute in emission order, so
    scores/exp/PV/normalize chunks are emitted BETWEEN projection chunks.
    The exp stream starts ~20us into the kernel instead of ~90us, the PE
    stays dense (no HAM clock-throttle oscillation), and the DVE chains
    trail the exp stream instead of serializing after it.
  - Softmax reciprocal chain pair-batched (one reciprocal/cast/broadcast per
    head pair); the partition broadcast of 1/denom runs on the otherwise-idle
    GpSimd engine instead of PE-matmul + ScalarE-copy.
  - wo unscaled; the 1/16 head-mean factor is folded into the host gather.
  - normed output and the x-residual are carried bf16 (halves those DMAs).
"""


import sys
import numpy as np

_REPO = "/opt/trn_rl_repo"
if _REPO not in sys.path:
    sys.path.insert(0, _REPO)

D = 1024       # d_model
S = 1024       # sequence length
B = 4          # batch
H = 16         # heads
DK = 64        # head dim
HALF = 512     # query rows per core
N_CORES = 8
LN_EPS = 1e-5

_cache = {}
GPSIMD_ADDS = 0   # number of ASUM ADD ops offloaded to GpSimd (0-14)


def _build_nc(ln_trivial=True):
    import concourse.bacc as bacc
    import concourse.mybir as mybir
    import concourse.tile as tile
    import concourse.bass as bass

    f32 = mybir.dt.float32
    bf16 = mybir.dt.bfloat16
    ADD = mybir.AluOpType.add
    MULT = mybir.AluOpType.mult

    nc = bacc.Bacc(None, target_bir_lowering=False)

    # ---- DRAM parameters (per-core views, SPMD-identical program) ----
    xT_p = nc.declare_dram_parameter("xT", [D, S], bf16, isOutput=False)      # x[b].T, query-half-first token order
    xr_p = nc.declare_dram_parameter("xr", [HALF, D], bf16, isOutput=False)    # x rows of our queries + o_bias
    wq_p = nc.declare_dram_parameter("wq", [D, D], bf16, isOutput=False)      # (q_w.T + 2 qA@qB)/8
    wk_p = nc.declare_dram_parameter("wk", [D, D], bf16, isOutput=False)      # k_w.T + 2 kA@kB
    wv_p = nc.declare_dram_parameter("wv", [D, D], bf16, isOutput=False)      # v_w.T + 2 vA@vB
    wo_p = nc.declare_dram_parameter("wo", [D, D], bf16, isOutput=False)      # o_w.T + 2 oA@oB
    bqa_p = nc.declare_dram_parameter("bqa", [128, 8], f32, isOutput=False)   # (q_b/8) arranged [p, ot]
    bv_p = nc.declare_dram_parameter("bv", [D], f32, isOutput=False)          # v_b
    lng_p = nc.declare_dram_parameter("lng", [D], f32, isOutput=False)        # ln gamma
    lnb_p = nc.declare_dram_parameter("lnb", [D], f32, isOutput=False)        # ln beta
    normed_p = nc.declare_dram_parameter("normed", [HALF, D], bf16, isOutput=True)
    attn_pa = nc.declare_dram_parameter("attn_out_a", [S, HALF], bf16, isOutput=True)  # sum_{h<8} attn_h, [k, q]
    attn_pb = nc.declare_dram_parameter("attn_out_b", [S, HALF], bf16, isOutput=True)  # sum_{h>=8} attn_h, [k, q]

    def bcast_ap(handle, dims):
        # broadcast a [D] dram vector across 128 partitions; dims shapes the free side
        ap = handle.ap()
        return bass.AP(tensor=ap.tensor, offset=ap.offset, ap=[[0, 128]] + dims)

    with tile.TileContext(nc) as tc:
        with (
            tc.tile_pool(name="consts", bufs=1) as consts,
            tc.tile_pool(name="res", bufs=1) as res,
            tc.tile_pool(name="xr", bufs=1) as xrpool,
        ):
            BQ = consts.tile([128, 8], f32, tag="bq")
            nc.sync.dma_start(out=BQ, in_=bqa_p[:, :])
            BV = consts.tile([128, 16, 64], f32, tag="bv")
            nc.sync.dma_start(out=BV, in_=bcast_ap(bv_p, [[64, 16], [1, 64]]))
            if not ln_trivial:
                GLN = consts.tile([128, 1024], f32, tag="gln")
                nc.sync.dma_start(out=GLN, in_=bcast_ap(lng_p, [[1, 1024]]))
                BLN = consts.tile([128, 1024], f32, tag="bln")
                nc.sync.dma_start(out=BLN, in_=bcast_ap(lnb_p, [[1, 1024]]))
            EPS = consts.tile([128, 1], f32, tag="eps")
            nc.vector.memset(EPS, LN_EPS)

            # ---------------- persistent activations ----------------
            KT_t = [res.tile([128, S], bf16, tag=f"KT{ot}", name=f"KT{ot}") for ot in range(8)]
            QT_t = [res.tile([128, HALF], bf16, tag=f"QT{ot}", name=f"QT{ot}") for ot in range(8)]
            VG_t = [res.tile([128, 16, 65], bf16, tag=f"VG{tt}", name=f"VG{tt}") for tt in range(8)]
            CTX_t = [res.tile([128, HALF], bf16, tag=f"CTX{ot}", name=f"CTX{ot}") for ot in range(8)]
            ASUM_A = res.tile([128, 8, HALF], bf16, tag="ASUMA")
            ASUM_B = res.tile([128, 8, HALF], bf16, tag="ASUMB")

            # ======== projections + attention, interleaved emission ========
            # Engine queues execute in emission order, so attention chunks are
            # emitted BETWEEN projection chunks: exp starts ~22us in, PE stays
            # dense (HAM warm), and the DVE chains trail the exp stream.
            xpool_cm = tc.tile_pool(name="xt", bufs=1)
            xpool = xpool_cm.__enter__()
            wpool_cm = tc.tile_pool(name="wts", bufs=16)
            wpool = wpool_cm.__enter__()
            apool_cm = tc.tile_pool(name="acc", bufs=2, space="PSUM")
            apool = apool_cm.__enter__()
            spool_cm = tc.tile_pool(name="sps", bufs=2, space="PSUM")
            spool = spool_cm.__enter__()
            cpool_cm = tc.tile_pool(name="cps", bufs=2, space="PSUM")
            cpool = cpool_cm.__enter__()
            epool_cm = tc.tile_pool(name="exp", bufs=2)
            epool = epool_cm.__enter__()
            rpool_cm = tc.tile_pool(name="rcp", bufs=2)
            rpool = rpool_cm.__enter__()
            rbpool_cm = tc.tile_pool(name="rbc", bufs=3)
            rbpool = rbpool_cm.__enter__()
            napool_cm = tc.tile_pool(name="tmp", bufs=2)
            napool = napool_cm.__enter__()

            XT = xpool.tile([128, 8, S], bf16, tag="XT")

            def load_w(param):
                tiles = []
                for it in range(8):
                    t = wpool.tile([128, 1024], bf16, tag="w")
                    nc.sync.dma_start(out=t, in_=param[it * 128:(it + 1) * 128, :])
                    tiles.append(t)
                return tiles

            wq_t = []
            for it in range(8):
                t = wpool.tile([128, 1024], bf16, tag="w")
                nc.sync.dma_start(out=t, in_=wq_p[it * 128:(it + 1) * 128, :])
                wq_t.append(t)
                nc.sync.dma_start(out=XT[:, it, 0:HALF],
                                  in_=xT_p[it * 128:(it + 1) * 128, 0:HALF])
            for it in range(8):
                nc.sync.dma_start(out=XT[:, it, HALF:S],
                                  in_=xT_p[it * 128:(it + 1) * 128, HALF:S])
            wk_t = load_w(wk_p)
            wv_t = []

            for tt in range(8):
                nc.vector.memset(VG_t[tt][:, :, 64:65], 1.0)

            # ---- emitters ----
            def emit_qproj():
                for ot in range(8):
                    ps = apool.tile([128, HALF], f32, tag="acc")
                    for it in range(8):
                        nc.tensor.matmul(
                            ps,
                            lhsT=wq_t[it][:, ot * 128:(ot + 1) * 128],
                            rhs=XT[:, it, 0:HALF],
                            start=(it == 0), stop=(it == 7),
                        )
                    nc.scalar.add(QT_t[ot], ps, BQ[:, ot:ot + 1])

            def emit_kproj(ot):
                psA = apool.tile([128, HALF], f32, tag="acc", name=f"kpsA{ot}")
                psB = apool.tile([128, HALF], f32, tag="acc", name=f"kpsB{ot}")
                for it in range(8):
                    for ncr, ps in ((0, psA), (1, psB)):
                        nc.tensor.matmul(
                            ps,
                            lhsT=wk_t[it][:, ot * 128:(ot + 1) * 128],
                            rhs=XT[:, it, ncr * 512:(ncr + 1) * 512],
                            start=(it == 0), stop=(it == 7),
                        )
                nc.scalar.copy(KT_t[ot][:, 0:512], psA)
                nc.scalar.copy(KT_t[ot][:, 512:1024], psB)

            def emit_vproj(ncr):
                for tt in range(8):
                    ps = apool.tile([128, HALF], f32, tag="acc")
                    for it in range(8):
                        nc.tensor.matmul(
                            ps,
                            lhsT=XT[:, it, tt * 128:(tt + 1) * 128],
                            rhs=wv_t[it][:, ncr * 512:(ncr + 1) * 512],
                            start=(it == 0), stop=(it == 7),
                        )
                    ps3 = ps.rearrange("p (h c) -> p h c", c=64)
                    nc.vector.tensor_tensor(
                        VG_t[tt][:, ncr * 8:(ncr + 1) * 8, 0:64], ps3,
                        BV[:, ncr * 8:(ncr + 1) * 8, :], ADD)

            def emit_scores(pr):
                EXPp = epool.tile([128, 2, 8, HALF], bf16, tag="exp", name=f"EXP{pr}")
                for kt in range(8):
                    sp = spool.tile([128, 2, HALF], f32, tag="sps")
                    for hh in range(2):
                        nc.tensor.matmul(
                            sp[:, hh, :],
                            lhsT=KT_t[pr][hh * 64:hh * 64 + 64, kt * 128:(kt + 1) * 128],
                            rhs=QT_t[pr][hh * 64:hh * 64 + 64, :],
                            start=True, stop=True,
                        )
                    nc.scalar.activation(
                        EXPp[:, :, kt, :], sp,
                        mybir.ActivationFunctionType.Exp,
                    )
                return EXPp

            def emit_pvc(pr, EXPp):
                # PV both heads, pair-batched reciprocal chain, CTX + ASUM
                cps = []
                dcp = rpool.tile([1, 2, HALF], f32, tag="dcp", name=f"dcp{pr}")
                for hh in range(2):
                    h = 2 * pr + hh
                    cp = cpool.tile([65, HALF], f32, tag="cps")
                    for kt in range(8):
                        nc.tensor.matmul(
                            cp,
                            lhsT=VG_t[kt][:, h, :],
                            rhs=EXPp[:, hh, kt, :],
                            start=(kt == 0), stop=(kt == 7),
                        )
                    nc.scalar.copy(dcp[:, hh, :], cp[64:65, :])
                    cps.append(cp)

                rec = rpool.tile([1, 2, HALF], f32, tag="rec", name=f"rec{pr}")
                nc.vector.reciprocal_approx_fast(
                    out=rec.rearrange("p a b -> p (a b)"),
                    in_=dcp.rearrange("p a b -> p (a b)"))
                rec_bf = rpool.tile([1, 2, HALF], bf16, tag="recbf", name=f"recbf{pr}")
                nc.vector.tensor_copy(
                    rec_bf.rearrange("p a b -> p (a b)"),
                    rec.rearrange("p a b -> p (a b)"))
                rbc = rbpool.tile([128, 2, HALF], bf16, tag="rbc", name=f"rbc{pr}")
                nc.gpsimd.partition_broadcast(
                    rbc.rearrange("p a b -> p (a b)"),
                    rec_bf.rearrange("p a b -> p (a b)")[0:1, :], channels=128)

                for hh in range(2):
                    h = 2 * pr + hh
                    cp = cps[hh]
                    nc.vector.tensor_tensor(
                        CTX_t[pr][hh * 64:hh * 64 + 64, :], cp[0:64, :],
                        rbc[0:64, hh, :], MULT,
                    )
                    ASUM = ASUM_A if h < 8 else ASUM_B
                    rbc_ap = rbc[:, hh, :]
                    rbc_w = bass.AP(tensor=rbc_ap.tensor, offset=rbc_ap.offset,
                                    ap=[rbc_ap.ap[0], [0, 8], rbc_ap.ap[1]])
                    if h % 8 == 0:
                        nc.vector.tensor_tensor(ASUM[:, :, :], EXPp[:, hh, :, :], rbc_w, MULT)
                    else:
                        tmpw = napool.tile([128, 8, HALF], bf16, tag="nrmattn")
                        nc.vector.tensor_tensor(tmpw, EXPp[:, hh, :, :], rbc_w, MULT)
                        nc.vector.tensor_tensor(ASUM[:, :, :], tmpw, ASUM[:, :, :], ADD)

            def ship(asum, attn_p):
                nc.sync.dma_start(
                    out=bass.AP(
                        tensor=attn_p.ap().tensor, offset=0,
                        ap=[[HALF, 128], [128 * HALF, 8], [1, HALF]],
                    ),
                    in_=asum[:, :, :],
                )

            # ---- interleaved emission ----
            emit_qproj()
            wv_t.extend(load_w(wv_p))
            emit_kproj(0)
            emit_kproj(1)
            exps = {}
            exps[0] = emit_scores(0)
            emit_vproj(0)
            exps[1] = emit_scores(1)
            emit_pvc(0, exps.pop(0))
            emit_kproj(2)
            emit_kproj(3)
            exps[2] = emit_scores(2)
            emit_pvc(1, exps.pop(1))
            emit_vproj(1)
            exps[3] = emit_scores(3)
            emit_pvc(2, exps.pop(2))
            emit_kproj(4)
            emit_kproj(5)
            exps[4] = emit_scores(4)
            emit_pvc(3, exps.pop(3))
            ship(ASUM_A, attn_pa)
            emit_kproj(6)
            emit_kproj(7)
            exps[5] = emit_scores(5)
            emit_pvc(4, exps.pop(4))
            exps[6] = emit_scores(6)
            emit_pvc(5, exps.pop(5))
            exps[7] = emit_scores(7)
            emit_pvc(6, exps.pop(6))
            emit_pvc(7, exps.pop(7))
            ship(ASUM_B, attn_pb)

            XR = xrpool.tile([128, 4, 1024], bf16, tag="XR")
            for tt in range(4):
                nc.sync.dma_start(out=XR[:, tt, :], in_=xr_p[tt * 128:(tt + 1) * 128, :])

            for cm in (napool_cm, rbpool_cm, rpool_cm, epool_cm, cpool_cm,
                       spool_cm, apool_cm, wpool_cm, xpool_cm):
                cm.__exit__(None, None, None)

            # wo load (scoped after attention pools close; ~6us before O-proj)
            wopool_cm = tc.tile_pool(name="wo2", bufs=8)
            wopool = wopool_cm.__enter__()
            wo_t = []
            for it in range(8):
                t = wopool.tile([128, 1024], bf16, tag="wo")
                nc.sync.dma_start(out=t, in_=wo_p[it * 128:(it + 1) * 128, :])
                wo_t.append(t)

            # ======== output projection + residual + LayerNorm ========
            with (
                tc.tile_pool(name="ln", bufs=2) as lpool,
                tc.tile_pool(name="ops", bufs=2, space="PSUM") as opool,
            ):

                for tt in range(4):
                    hh_t = lpool.tile([128, 1024], f32, tag="hh")
                    for ncr in range(2):
                        ps = opool.tile([128, HALF], f32, tag="ops")
                        for it in range(8):
                            nc.tensor.matmul(
                                ps,
                                lhsT=CTX_t[it][:, tt * 128:(tt + 1) * 128],
                                rhs=wo_t[it][:, ncr * 512:(ncr + 1) * 512],
                                start=(it == 0), stop=(it == 7),
                            )
                        nc.vector.tensor_tensor(
                            hh_t[:, ncr * 512:(ncr + 1) * 512], ps,
                            XR[:, tt, ncr * 512:(ncr + 1) * 512], ADD)

                    st = lpool.tile([128, 2, 6], f32, tag="st")
                    for g2 in range(2):
                        nc.vector.bn_stats(st[:, g2, :], hh_t[:, g2 * 512:(g2 + 1) * 512])
                    mv = lpool.tile([128, 2], f32, tag="mv")
                    nc.vector.bn_aggr(mv, st)
                    nmu = lpool.tile([128, 1], f32, tag="nmu")
                    nc.vector.tensor_scalar_mul(nmu, mv[:, 0:1], -1.0)
                    sd = lpool.tile([128, 1], f32, tag="sd")
                    nc.scalar.activation(
                        sd, mv[:, 1:2], mybir.ActivationFunctionType.Sqrt,
                        bias=EPS[:, 0:1], scale=1.0,
                    )
                    rstd = lpool.tile([128, 1], f32, tag="rstd")
                    nc.vector.reciprocal(rstd, sd)

                    t1 = lpool.tile([128, 1024], bf16 if ln_trivial else f32, tag="t1")
                    nc.vector.tensor_scalar(t1, hh_t, nmu[:, 0:1], rstd[:, 0:1], ADD, MULT)
                    if not ln_trivial:
                        t2 = lpool.tile([128, 1024], f32, tag="t2")
                        nc.vector.tensor_tensor(t2, t1, GLN, MULT)
                        nrm = lpool.tile([128, 1024], bf16, tag="nrm")
                        nc.vector.tensor_tensor(nrm, t2, BLN, ADD)
                        t1 = nrm
                    nc.sync.dma_start(out=normed_p[tt * 128:(tt + 1) * 128, :], in_=t1)

            wopool_cm.__exit__(None, None, None)

    nc.finalize()
    return nc


def _get_nc(ln_trivial=True):
    key = f"nc_{ln_trivial}"
    if key not in _cache:
        _cache[key] = _build_nc(ln_trivial=ln_trivial)
    return _cache[key]


def _prep_in_maps(inputs):
    x = np.asarray(inputs["x"], dtype=np.float32)
    w_eff = {}
    for p in ("q", "k", "v", "o"):
        w = np.asarray(inputs[f"{p}_w"], dtype=np.float32)
        A = np.asarray(inputs[f"{p}_A"], dtype=np.float32)
        Bm = np.asarray(inputs[f"{p}_B"], dtype=np.float32)
        w_eff[p] = w.T + 2.0 * (A @ Bm)          # [in, out]
    import ml_dtypes
    bf = ml_dtypes.bfloat16
    wq = (w_eff["q"] / 8.0).astype(bf)
    wk = w_eff["k"].astype(bf)
    wv = w_eff["v"].astype(bf)
    wo = w_eff["o"].astype(bf)
    bqa = (np.asarray(inputs["q_b"], np.float32) / 8.0).reshape(8, 128).T.copy()
    bv = np.ascontiguousarray(inputs["v_b"], dtype=np.float32)
    ob = np.asarray(inputs["o_b"], np.float32)
    lng = np.ascontiguousarray(inputs["ln_g"], dtype=np.float32)
    lnb = np.ascontiguousarray(inputs["ln_b"], dtype=np.float32)

    in_maps = []
    for c in range(N_CORES):
        b, qh = c // 2, c % 2
        xb = x[b]                                  # [S, D]
        xT = np.ascontiguousarray(xb.T)            # [D, S]
        if qh == 1:
            xT = np.concatenate([xT[:, HALF:], xT[:, :HALF]], axis=1)
        xT = xT.astype(bf)
        xr = np.ascontiguousarray(xb[qh * HALF:(qh + 1) * HALF, :] + ob[None, :]).astype(bf)
        in_maps.append({
            "xT": xT, "xr": xr,
            "wq": wq, "wk": wk, "wv": wv, "wo": wo,
            "bqa": bqa, "bv": bv,
            "lng": lng, "lnb": lnb,
        })
    return in_maps


def run_on_device(inputs, trace=False, tmpdir=None):
    from concourse.bass_utils import run_bass_kernel_spmd

    ln_trivial = bool(
        np.all(np.asarray(inputs["ln_g"]) == 1.0)
        and np.all(np.asarray(inputs["ln_b"]) == 0.0))
    nc = _get_nc(ln_trivial=ln_trivial)
    in_maps = _prep_in_maps(inputs)
    res = run_bass_kernel_spmd(
        nc, in_maps, core_ids=list(range(N_CORES)), trace=trace, tmpdir=tmpdir,
    )

    normed = np.zeros((B, S, D), dtype=np.float32)
    attn_mean = np.zeros((B, S, S), dtype=np.float32)
    for c in range(N_CORES):
        b, qh = c // 2, c % 2
        normed[b, qh * HALF:(qh + 1) * HALF, :] = np.asarray(
            res.results[c]["normed"], dtype=np.float32)
    for b in range(B):
        halves = []
        for qh in range(2):
            r = res.results[2 * b + qh]
            A = (np.asarray(r["attn_out_a"], dtype=np.float32)
                 + np.asarray(r["attn_out_b"], dtype=np.float32)) * (1.0 / 16.0)
            if qh == 1:
                A = np.concatenate([A[HALF:], A[:HALF]], axis=0)  # undo k-perm
            halves.append(A)                       # [S(k), HALF(q)]
        attn_mean[b] = np.concatenate(halves, axis=1).T
    return (normed, attn_mean), res


def kernel(**inputs):
    (normed, attn_mean), _ = run_on_device(inputs, trace=False)
    return normed, attn_mean


# revision 3
# speedup vs baseline: 1.1412x; 1.1412x over previous
"""Trainium2 Bass kernel for EnhancedMultiHeadAttention (LoRA MHA + residual + LayerNorm).

Contract: kernel(**inputs) takes the FULL unsharded inputs (as produced by
setup_inputs()) and returns the full outputs (normed, attn.mean(axis=1)).

Sharding: 8 cores = 4 batches x 2 query-halves. Each core computes K/V for the
whole sequence of its batch (duplicated across the pair) and attention +
output-projection + LayerNorm for its 512 query rows.

Key optimizations (231us -> ~213us):
  - K bias dropped entirely: softmax over k is invariant to the per-q constant
    q.k_b adds to scores, so neither output depends on it.
  - Interleaved emission: engine queues execute in emission order, so
    scores/exp/PV/normalize chunks are emitted BETWEEN projection chunks.
    The exp stream starts early, the PE stays dense (no HAM clock-throttle
    oscillation mid-kernel), and the DVE chains trail the exp stream.
  - Coarse DMA descriptors (2 per weight / x^T half): the serialized
    descriptor issue on the Sync engine was delaying the K projection and
    with it the whole attention pipeline by ~15us.
  - Softmax reciprocal chain pair-batched; the 1/denom partition-broadcast
    runs on the otherwise-idle GpSimd engine instead of PE+ScalarE.
  - wo unscaled; the 1/16 head-mean factor folded into the host gather;
    normed output and the x-residual carried bf16.
"""



import sys
import numpy as np

_REPO = "/opt/trn_rl_repo"
if _REPO not in sys.path:
    sys.path.insert(0, _REPO)

D = 1024       # d_model
S = 1024       # sequence length
B = 4          # batch
H = 16         # heads
DK = 64        # head dim
HALF = 512     # query rows per core
N_CORES = 8
LN_EPS = 1e-5

_cache = {}
GPSIMD_ADDS = 0   # number of ASUM ADD ops offloaded to GpSimd (0-14)


def _build_nc(ln_trivial=True):
    import concourse.bacc as bacc
    import concourse.mybir as mybir
    import concourse.tile as tile
    import concourse.bass as bass

    f32 = mybir.dt.float32
    bf16 = mybir.dt.bfloat16
    ADD = mybir.AluOpType.add
    MULT = mybir.AluOpType.mult

    nc = bacc.Bacc(None, target_bir_lowering=False)

    # ---- DRAM parameters (per-core views, SPMD-identical program) ----
    xT_p = nc.declare_dram_parameter("xT", [D, S], bf16, isOutput=False)      # x[b].T, query-half-first token order
    xr_p = nc.declare_dram_parameter("xr", [HALF, D], bf16, isOutput=False)    # x rows of our queries + o_bias
    wq_p = nc.declare_dram_parameter("wq", [D, D], bf16, isOutput=False)      # (q_w.T + 2 qA@qB)/8
    wk_p = nc.declare_dram_parameter("wk", [D, D], bf16, isOutput=False)      # k_w.T + 2 kA@kB
    wv_p = nc.declare_dram_parameter("wv", [D, D], bf16, isOutput=False)      # v_w.T + 2 vA@vB
    wo_p = nc.declare_dram_parameter("wo", [D, D], bf16, isOutput=False)      # o_w.T + 2 oA@oB
    bqa_p = nc.declare_dram_parameter("bqa", [128, 8], f32, isOutput=False)   # (q_b/8) arranged [p, ot]
    bv_p = nc.declare_dram_parameter("bv", [D], f32, isOutput=False)          # v_b
    lng_p = nc.declare_dram_parameter("lng", [D], f32, isOutput=False)        # ln gamma
    lnb_p = nc.declare_dram_parameter("lnb", [D], f32, isOutput=False)        # ln beta
    normed_p = nc.declare_dram_parameter("normed", [HALF, D], bf16, isOutput=True)
    attn_pa = nc.declare_dram_parameter("attn_out_a", [S, HALF], bf16, isOutput=True)  # sum_{h<8} attn_h, [k, q]
    attn_pb = nc.declare_dram_parameter("attn_out_b", [S, HALF], bf16, isOutput=True)  # sum_{h>=8} attn_h, [k, q]

    def bcast_ap(handle, dims):
        # broadcast a [D] dram vector across 128 partitions; dims shapes the free side
        ap = handle.ap()
        return bass.AP(tensor=ap.tensor, offset=ap.offset, ap=[[0, 128]] + dims)

    with tile.TileContext(nc) as tc:
        with (
            tc.tile_pool(name="consts", bufs=1) as consts,
            tc.tile_pool(name="res", bufs=1) as res,
        ):
            BQ = consts.tile([128, 8], f32, tag="bq")
            nc.sync.dma_start(out=BQ, in_=bqa_p[:, :])
            BV = consts.tile([128, 16, 64], f32, tag="bv")
            nc.sync.dma_start(out=BV, in_=bcast_ap(bv_p, [[64, 16], [1, 64]]))
            if not ln_trivial:
                GLN = consts.tile([128, 1024], f32, tag="gln")
                nc.sync.dma_start(out=GLN, in_=bcast_ap(lng_p, [[1, 1024]]))
                BLN = consts.tile([128, 1024], f32, tag="bln")
                nc.sync.dma_start(out=BLN, in_=bcast_ap(lnb_p, [[1, 1024]]))
            EPS = consts.tile([128, 1], f32, tag="eps")
            nc.vector.memset(EPS, LN_EPS)

            # ---------------- persistent activations ----------------
            KT_t = [res.tile([128, S], bf16, tag=f"KT{ot}", name=f"KT{ot}") for ot in range(8)]
            QT_t = [res.tile([128, HALF], bf16, tag=f"QT{ot}", name=f"QT{ot}") for ot in range(8)]
            VG_t = [res.tile([128, 16, 65], bf16, tag=f"VG{tt}", name=f"VG{tt}") for tt in range(8)]
            CTX_t = [res.tile([128, HALF], bf16, tag=f"CTX{ot}", name=f"CTX{ot}") for ot in range(8)]
            ASUM_A = res.tile([128, 8, HALF], bf16, tag="ASUMA")
            ASUM_B = res.tile([128, 8, HALF], bf16, tag="ASUMB")

            # ======== projections + attention, interleaved emission ========
            # Engine queues execute in emission order, so attention chunks are
            # emitted BETWEEN projection chunks: exp starts ~22us in, PE stays
            # dense (HAM warm), and the DVE chains trail the exp stream.
            xpool_cm = tc.tile_pool(name="xt", bufs=1)
            xpool = xpool_cm.__enter__()
            wpool_cm = tc.tile_pool(name="wts", bufs=2)
            wpool = wpool_cm.__enter__()
            apool_cm = tc.tile_pool(name="acc", bufs=2, space="PSUM")
            apool = apool_cm.__enter__()
            spool_cm = tc.tile_pool(name="sps", bufs=2, space="PSUM")
            spool = spool_cm.__enter__()
            cpool_cm = tc.tile_pool(name="cps", bufs=2, space="PSUM")
            cpool = cpool_cm.__enter__()
            epool_cm = tc.tile_pool(name="exp", bufs=3)
            epool = epool_cm.__enter__()
            rpool_cm = tc.tile_pool(name="rcp", bufs=2)
            rpool = rpool_cm.__enter__()
            rbpool_cm = tc.tile_pool(name="rbc", bufs=3)
            rbpool = rbpool_cm.__enter__()
            napool_cm = tc.tile_pool(name="tmp", bufs=2)
            napool = napool_cm.__enter__()

            XT = xpool.tile([128, 8, S], bf16, tag="XT")

            def load_w(param):
                # one [128, 8, 1024] tile, 2 descriptors (chains read all 8
                # sub-tiles anyway, so coarse descs cost no pipelining)
                t = wpool.tile([128, 8, 1024], bf16, tag="w", name="wtile")
                for j in range(2):
                    src_ap = bass.AP(
                        tensor=param.ap().tensor,
                        offset=j * 4 * 128 * 1024,
                        ap=[[1024, 128], [128 * 1024, 4], [1, 1024]],
                    )
                    nc.sync.dma_start(out=t[:, j * 4:(j + 1) * 4, :], in_=src_ap)
                return t

            def load_xt(half):
                for j in range(2):
                    src_ap = bass.AP(
                        tensor=xT_p.ap().tensor,
                        offset=j * 4 * 128 * 1024 + half * 512,
                        ap=[[1024, 128], [128 * 1024, 4], [1, 512]],
                    )
                    nc.sync.dma_start(
                        out=XT[:, j * 4:(j + 1) * 4, half * 512:(half + 1) * 512],
                        in_=src_ap)

            wq_t = load_w(wq_p)
            load_xt(0)
            wk_t = load_w(wk_p)
            load_xt(1)
            wv_t = None

            for tt in range(8):
                nc.vector.memset(VG_t[tt][:, :, 64:65], 1.0)

            # ---- emitters ----
            def emit_qproj():
                for ot in range(8):
                    ps = apool.tile([128, HALF], f32, tag="acc")
                    for it in range(8):
                        nc.tensor.matmul(
                            ps,
                            lhsT=wq_t[:, it, ot * 128:(ot + 1) * 128],
                            rhs=XT[:, it, 0:HALF],
                            start=(it == 0), stop=(it == 7),
                        )
                    nc.scalar.add(QT_t[ot], ps, BQ[:, ot:ot + 1])

            def emit_kproj(ot):
                psA = apool.tile([128, HALF], f32, tag="acc", name=f"kpsA{ot}")
                psB = apool.tile([128, HALF], f32, tag="acc", name=f"kpsB{ot}")
                for it in range(8):
                    for ncr, ps in ((0, psA), (1, psB)):
                        nc.tensor.matmul(
                            ps,
                            lhsT=wk_t[:, it, ot * 128:(ot + 1) * 128],
                            rhs=XT[:, it, ncr * 512:(ncr + 1) * 512],
                            start=(it == 0), stop=(it == 7),
                        )
                nc.scalar.copy(KT_t[ot][:, 0:512], psA)
                nc.scalar.copy(KT_t[ot][:, 512:1024], psB)

            def emit_vproj(ncr):
                for tt in range(8):
                    ps = apool.tile([128, HALF], f32, tag="acc")
                    for it in range(8):
                        nc.tensor.matmul(
                            ps,
                            lhsT=XT[:, it, tt * 128:(tt + 1) * 128],
                            rhs=wv_t[:, it, ncr * 512:(ncr + 1) * 512],
                            start=(it == 0), stop=(it == 7),
                        )
                    ps3 = ps.rearrange("p (h c) -> p h c", c=64)
                    nc.vector.tensor_tensor(
                        VG_t[tt][:, ncr * 8:(ncr + 1) * 8, 0:64], ps3,
                        BV[:, ncr * 8:(ncr + 1) * 8, :], ADD)

            def emit_scores(pr):
                EXPp = epool.tile([128, 2, 8, HALF], bf16, tag="exp", name=f"EXP{pr}")
                for kt in range(8):
                    sp = spool.tile([128, 2, HALF], f32, tag="sps")
                    for hh in range(2):
                        nc.tensor.matmul(
                            sp[:, hh, :],
                            lhsT=KT_t[pr][hh * 64:hh * 64 + 64, kt * 128:(kt + 1) * 128],
                            rhs=QT_t[pr][hh * 64:hh * 64 + 64, :],
                            start=True, stop=True,
                        )
                    nc.scalar.activation(
                        EXPp[:, :, kt, :], sp,
                        mybir.ActivationFunctionType.Exp,
                    )
                return EXPp

            def emit_pvc(pr, EXPp):
                # PV both heads, pair-batched reciprocal chain, CTX + ASUM
                cps = []
                dcp = rpool.tile([1, 2, HALF], f32, tag="dcp", name=f"dcp{pr}")
                for hh in range(2):
                    h = 2 * pr + hh
                    cp = cpool.tile([65, HALF], f32, tag="cps")
                    for kt in range(8):
                        nc.tensor.matmul(
                            cp,
                            lhsT=VG_t[kt][:, h, :],
                            rhs=EXPp[:, hh, kt, :],
                            start=(kt == 0), stop=(kt == 7),
                        )
                    nc.scalar.copy(dcp[:, hh, :], cp[64:65, :])
                    cps.append(cp)

                rec = rpool.tile([1, 2, HALF], f32, tag="rec", name=f"rec{pr}")
                nc.vector.reciprocal_approx_fast(
                    out=rec.rearrange("p a b -> p (a b)"),
                    in_=dcp.rearrange("p a b -> p (a b)"))
                rec_bf = rpool.tile([1, 2, HALF], bf16, tag="recbf", name=f"recbf{pr}")
                nc.vector.tensor_copy(
                    rec_bf.rearrange("p a b -> p (a b)"),
                    rec.rearrange("p a b -> p (a b)"))
                rbc = rbpool.tile([128, 2, HALF], bf16, tag="rbc", name=f"rbc{pr}")
                nc.gpsimd.partition_broadcast(
                    rbc.rearrange("p a b -> p (a b)"),
                    rec_bf.rearrange("p a b -> p (a b)")[0:1, :], channels=128)

                for hh in range(2):
                    h = 2 * pr + hh
                    cp = cps[hh]
                    nc.vector.tensor_tensor(
                        CTX_t[pr][hh * 64:hh * 64 + 64, :], cp[0:64, :],
                        rbc[0:64, hh, :], MULT,
                    )
                    ASUM = ASUM_A if h < 8 else ASUM_B
                    rbc_ap = rbc[:, hh, :]
                    rbc_w = bass.AP(tensor=rbc_ap.tensor, offset=rbc_ap.offset,
                                    ap=[rbc_ap.ap[0], [0, 8], rbc_ap.ap[1]])
                    if h % 8 == 0:
                        nc.vector.tensor_tensor(ASUM[:, :, :], EXPp[:, hh, :, :], rbc_w, MULT)
                    else:
                        tmpw = napool.tile([128, 8, HALF], bf16, tag="nrmattn")
                        nc.vector.tensor_tensor(tmpw, EXPp[:, hh, :, :], rbc_w, MULT)
                        nc.vector.tensor_tensor(ASUM[:, :, :], tmpw, ASUM[:, :, :], ADD)

            def ship(asum, attn_p):
                nc.sync.dma_start(
                    out=bass.AP(
                        tensor=attn_p.ap().tensor, offset=0,
                        ap=[[HALF, 128], [128 * HALF, 8], [1, HALF]],
                    ),
                    in_=asum[:, :, :],
                )

            # ---- interleaved emission ----
            emit_qproj()
            wv_t = load_w(wv_p)
            emit_kproj(0)
            emit_kproj(1)
            exps = {}
            exps[0] = emit_scores(0)
            emit_vproj(0)
            exps[1] = emit_scores(1)
            emit_pvc(0, exps.pop(0))
            emit_kproj(2)
            emit_kproj(3)
            exps[2] = emit_scores(2)
            emit_pvc(1, exps.pop(1))
            emit_vproj(1)
            exps[3] = emit_scores(3)
            emit_pvc(2, exps.pop(2))
            emit_kproj(4)
            emit_kproj(5)
            exps[4] = emit_scores(4)
            emit_pvc(3, exps.pop(3))
            ship(ASUM_A, attn_pa)
            emit_kproj(6)
            emit_kproj(7)
            exps[5] = emit_scores(5)
            emit_pvc(4, exps.pop(4))
            exps[6] = emit_scores(6)
            emit_pvc(5, exps.pop(5))
            exps[7] = emit_scores(7)
            emit_pvc(6, exps.pop(6))
            emit_pvc(7, exps.pop(7))
            ship(ASUM_B, attn_pb)

            for cm in (napool_cm, rbpool_cm, rpool_cm, epool_cm, cpool_cm,
                       spool_cm, apool_cm, wpool_cm, xpool_cm):
                cm.__exit__(None, None, None)

            # wo + XR load (scoped after attention pools close)
            wopool_cm = tc.tile_pool(name="wo2", bufs=1)
            wopool = wopool_cm.__enter__()
            wo_t = wopool.tile([128, 8, 1024], bf16, tag="wo", name="wo_t")
            for j in range(2):
                src_ap = bass.AP(
                    tensor=wo_p.ap().tensor,
                    offset=j * 4 * 128 * 1024,
                    ap=[[1024, 128], [128 * 1024, 4], [1, 1024]],
                )
                nc.sync.dma_start(out=wo_t[:, j * 4:(j + 1) * 4, :], in_=src_ap)
            xrpool_cm = tc.tile_pool(name="xr", bufs=1)
            xrpool = xrpool_cm.__enter__()
            XR = xrpool.tile([128, 4, 1024], bf16, tag="XR")
            nc.sync.dma_start(
                out=XR,
                in_=bass.AP(tensor=xr_p.ap().tensor, offset=0,
                            ap=[[1024, 128], [128 * 1024, 4], [1, 1024]]))

            # ======== output projection + residual + LayerNorm ========
            with (
                tc.tile_pool(name="ln", bufs=2) as lpool,
                tc.tile_pool(name="ops", bufs=2, space="PSUM") as opool,
            ):

                for tt in range(4):
                    hh_t = lpool.tile([128, 1024], f32, tag="hh")
                    for ncr in range(2):
                        ps = opool.tile([128, HALF], f32, tag="ops")
                        for it in range(8):
                            nc.tensor.matmul(
                                ps,
                                lhsT=CTX_t[it][:, tt * 128:(tt + 1) * 128],
                                rhs=wo_t[:, it, ncr * 512:(ncr + 1) * 512],
                                start=(it == 0), stop=(it == 7),
                            )
                        nc.vector.tensor_tensor(
                            hh_t[:, ncr * 512:(ncr + 1) * 512], ps,
                            XR[:, tt, ncr * 512:(ncr + 1) * 512], ADD)

                    st = lpool.tile([128, 2, 6], f32, tag="st")
                    for g2 in range(2):
                        nc.vector.bn_stats(st[:, g2, :], hh_t[:, g2 * 512:(g2 + 1) * 512])
                    mv = lpool.tile([128, 2], f32, tag="mv")
                    nc.vector.bn_aggr(mv, st)
                    nmu = lpool.tile([128, 1], f32, tag="nmu")
                    nc.vector.tensor_scalar_mul(nmu, mv[:, 0:1], -1.0)
                    sd = lpool.tile([128, 1], f32, tag="sd")
                    nc.scalar.activation(
                        sd, mv[:, 1:2], mybir.ActivationFunctionType.Sqrt,
                        bias=EPS[:, 0:1], scale=1.0,
                    )
                    rstd = lpool.tile([128, 1], f32, tag="rstd")
                    nc.vector.reciprocal(rstd, sd)

                    t1 = lpool.tile([128, 1024], bf16 if ln_trivial else f32, tag="t1")
                    nc.vector.tensor_scalar(t1, hh_t, nmu[:, 0:1], rstd[:, 0:1], ADD, MULT)
                    if not ln_trivial:
                        t2 = lpool.tile([128, 1024], f32, tag="t2")
                        nc.vector.tensor_tensor(t2, t1, GLN, MULT)
                        nrm = lpool.tile([128, 1024], bf16, tag="nrm")
                        nc.vector.tensor_tensor(nrm, t2, BLN, ADD)
                        t1 = nrm
                    nc.sync.dma_start(out=normed_p[tt * 128:(tt + 1) * 128, :], in_=t1)

            xrpool_cm.__exit__(None, None, None)
            wopool_cm.__exit__(None, None, None)

    nc.finalize()
    return nc


def _get_nc(ln_trivial=True):
    key = f"nc_{ln_trivial}"
    if key not in _cache:
        _cache[key] = _build_nc(ln_trivial=ln_trivial)
    return _cache[key]


def _prep_in_maps(inputs):
    x = np.asarray(inputs["x"], dtype=np.float32)
    w_eff = {}
    for p in ("q", "k", "v", "o"):
        w = np.asarray(inputs[f"{p}_w"], dtype=np.float32)
        A = np.asarray(inputs[f"{p}_A"], dtype=np.float32)
        Bm = np.asarray(inputs[f"{p}_B"], dtype=np.float32)
        w_eff[p] = w.T + 2.0 * (A @ Bm)          # [in, out]
    import ml_dtypes
    bf = ml_dtypes.bfloat16
    wq = (w_eff["q"] / 8.0).astype(bf)
    wk = w_eff["k"].astype(bf)
    wv = w_eff["v"].astype(bf)
    wo = w_eff["o"].astype(bf)
    bqa = (np.asarray(inputs["q_b"], np.float32) / 8.0).reshape(8, 128).T.copy()
    bv = np.ascontiguousarray(inputs["v_b"], dtype=np.float32)
    ob = np.asarray(inputs["o_b"], np.float32)
    lng = np.ascontiguousarray(inputs["ln_g"], dtype=np.float32)
    lnb = np.ascontiguousarray(inputs["ln_b"], dtype=np.float32)

    in_maps = []
    for c in range(N_CORES):
        b, qh = c // 2, c % 2
        xb = x[b]                                  # [S, D]
        xT = np.ascontiguousarray(xb.T)            # [D, S]
        if qh == 1:
            xT = np.concatenate([xT[:, HALF:], xT[:, :HALF]], axis=1)
        xT = xT.astype(bf)
        xr = np.ascontiguousarray(xb[qh * HALF:(qh + 1) * HALF, :] + ob[None, :]).astype(bf)
        in_maps.append({
            "xT": xT, "xr": xr,
            "wq": wq, "wk": wk, "wv": wv, "wo": wo,
            "bqa": bqa, "bv": bv,
            "lng": lng, "lnb": lnb,
        })
    return in_maps


def run_on_device(inputs, trace=False, tmpdir=None):
    from concourse.bass_utils import run_bass_kernel_spmd

    ln_trivial = bool(
        np.all(np.asarray(inputs["ln_g"]) == 1.0)
        and np.all(np.asarray(inputs["ln_b"]) == 0.0))
    nc = _get_nc(ln_trivial=ln_trivial)
    in_maps = _prep_in_maps(inputs)
    res = run_bass_kernel_spmd(
        nc, in_maps, core_ids=list(range(N_CORES)), trace=trace, tmpdir=tmpdir,
    )

    normed = np.zeros((B, S, D), dtype=np.float32)
    attn_mean = np.zeros((B, S, S), dtype=np.float32)
    for c in range(N_CORES):
        b, qh = c // 2, c % 2
        normed[b, qh * HALF:(qh + 1) * HALF, :] = np.asarray(
            res.results[c]["normed"], dtype=np.float32)
    for b in range(B):
        halves = []
        for qh in range(2):
            r = res.results[2 * b + qh]
            A = (np.asarray(r["attn_out_a"], dtype=np.float32)
                 + np.asarray(r["attn_out_b"], dtype=np.float32)) * (1.0 / 16.0)
            if qh == 1:
                A = np.concatenate([A[HALF:], A[:HALF]], axis=0)  # undo k-perm
            halves.append(A)                       # [S(k), HALF(q)]
        attn_mean[b] = np.concatenate(halves, axis=1).T
    return (normed, attn_mean), res


def kernel(**inputs):
    (normed, attn_mean), _ = run_on_device(inputs, trace=False)
    return normed, attn_mean


# revision 4
# speedup vs baseline: 1.1722x; 1.0272x over previous
"""Trainium2 Bass kernel for EnhancedMultiHeadAttention (LoRA MHA + residual + LayerNorm).

Contract: kernel(**inputs) takes the FULL unsharded inputs (as produced by
setup_inputs()) and returns the full outputs (normed, attn.mean(axis=1)).

Sharding: 8 cores = 4 batches x 2 query-halves. Each core computes K/V for the
whole sequence of its batch (duplicated across the pair) and attention +
output-projection + LayerNorm for its 512 query rows.

Key optimizations (231us -> ~213us):
  - K bias dropped entirely: softmax over k is invariant to the per-q constant
    q.k_b adds to scores, so neither output depends on it.
  - Interleaved emission: engine queues execute in emission order, so
    scores/exp/PV/normalize chunks are emitted BETWEEN projection chunks.
    The exp stream starts early, the PE stays dense (no HAM clock-throttle
    oscillation mid-kernel), and the DVE chains trail the exp stream.
  - Coarse DMA descriptors (2 per weight / x^T half): serialized descriptor
    issue on the Sync engine was delaying the K projection and with it the
    whole attention pipeline by ~15us.
  - Softmax reciprocal chain pair-batched; the 1/denom partition-broadcast
    runs on the otherwise-idle GpSimd engine instead of PE+ScalarE.
  - Attention-mean kept as 8 pair-level partial sums shipped straight to
    DRAM (host sums them): fewer serial DVE adds on the attention critical
    path and a shorter bf16 accumulation chain (better precision).
  - wo unscaled; the 1/16 head-mean factor folded into the host gather;
    normed output and the x-residual carried bf16.
"""



import sys
import numpy as np

_REPO = "/opt/trn_rl_repo"
if _REPO not in sys.path:
    sys.path.insert(0, _REPO)

D = 1024       # d_model
S = 1024       # sequence length
B = 4          # batch
H = 16         # heads
DK = 64        # head dim
HALF = 512     # query rows per core
N_CORES = 8
LN_EPS = 1e-5

_cache = {}
GPSIMD_ADDS = 0   # number of ASUM ADD ops offloaded to GpSimd (0-14)


def _build_nc(ln_trivial=True):
    import concourse.bacc as bacc
    import concourse.mybir as mybir
    import concourse.tile as tile
    import concourse.bass as bass

    f32 = mybir.dt.float32
    bf16 = mybir.dt.bfloat16
    ADD = mybir.AluOpType.add
    MULT = mybir.AluOpType.mult

    nc = bacc.Bacc(None, target_bir_lowering=False)

    # ---- DRAM parameters (per-core views, SPMD-identical program) ----
    xT_p = nc.declare_dram_parameter("xT", [D, S], bf16, isOutput=False)      # x[b].T, query-half-first token order
    xr_p = nc.declare_dram_parameter("xr", [HALF, D], bf16, isOutput=False)    # x rows of our queries + o_bias
    wq_p = nc.declare_dram_parameter("wq", [D, D], bf16, isOutput=False)      # (q_w.T + 2 qA@qB)/8
    wk_p = nc.declare_dram_parameter("wk", [D, D], bf16, isOutput=False)      # k_w.T + 2 kA@kB
    wv_p = nc.declare_dram_parameter("wv", [D, D], bf16, isOutput=False)      # v_w.T + 2 vA@vB
    wo_p = nc.declare_dram_parameter("wo", [D, D], bf16, isOutput=False)      # o_w.T + 2 oA@oB
    bqa_p = nc.declare_dram_parameter("bqa", [128, 8], f32, isOutput=False)   # (q_b/8) arranged [p, ot]
    bv_p = nc.declare_dram_parameter("bv", [D], f32, isOutput=False)          # v_b
    lng_p = nc.declare_dram_parameter("lng", [D], f32, isOutput=False)        # ln gamma
    lnb_p = nc.declare_dram_parameter("lnb", [D], f32, isOutput=False)        # ln beta
    normed_p = nc.declare_dram_parameter("normed", [HALF, D], bf16, isOutput=True)
    attn_p = nc.declare_dram_parameter("attn_out", [8, S, HALF], bf16, isOutput=True)  # per-pair attn sums, [pr, k, q]

    def bcast_ap(handle, dims):
        # broadcast a [D] dram vector across 128 partitions; dims shapes the free side
        ap = handle.ap()
        return bass.AP(tensor=ap.tensor, offset=ap.offset, ap=[[0, 128]] + dims)

    with tile.TileContext(nc) as tc:
        with (
            tc.tile_pool(name="consts", bufs=1) as consts,
            tc.tile_pool(name="res", bufs=1) as res,
        ):
            BQ = consts.tile([128, 8], f32, tag="bq")
            nc.sync.dma_start(out=BQ, in_=bqa_p[:, :])
            BV = consts.tile([128, 16, 64], f32, tag="bv")
            nc.sync.dma_start(out=BV, in_=bcast_ap(bv_p, [[64, 16], [1, 64]]))
            if not ln_trivial:
                GLN = consts.tile([128, 1024], f32, tag="gln")
                nc.sync.dma_start(out=GLN, in_=bcast_ap(lng_p, [[1, 1024]]))
                BLN = consts.tile([128, 1024], f32, tag="bln")
                nc.sync.dma_start(out=BLN, in_=bcast_ap(lnb_p, [[1, 1024]]))
            EPS = consts.tile([128, 1], f32, tag="eps")
            nc.vector.memset(EPS, LN_EPS)

            # ---------------- persistent activations ----------------
            KT_t = [res.tile([128, S], bf16, tag=f"KT{ot}", name=f"KT{ot}") for ot in range(8)]
            QT_t = [res.tile([128, HALF], bf16, tag=f"QT{ot}", name=f"QT{ot}") for ot in range(8)]
            VG_t = [res.tile([128, 16, 65], bf16, tag=f"VG{tt}", name=f"VG{tt}") for tt in range(8)]
            CTX_t = [res.tile([128, HALF], bf16, tag=f"CTX{ot}", name=f"CTX{ot}") for ot in range(8)]

            # ======== projections + attention, interleaved emission ========
            # Engine queues execute in emission order, so attention chunks are
            # emitted BETWEEN projection chunks: exp starts ~22us in, PE stays
            # dense (HAM warm), and the DVE chains trail the exp stream.
            xpool_cm = tc.tile_pool(name="xt", bufs=1)
            xpool = xpool_cm.__enter__()
            wpool_cm = tc.tile_pool(name="wts", bufs=2)
            wpool = wpool_cm.__enter__()
            apool_cm = tc.tile_pool(name="acc", bufs=2, space="PSUM")
            apool = apool_cm.__enter__()
            spool_cm = tc.tile_pool(name="sps", bufs=2, space="PSUM")
            spool = spool_cm.__enter__()
            cpool_cm = tc.tile_pool(name="cps", bufs=2, space="PSUM")
            cpool = cpool_cm.__enter__()
            epool_cm = tc.tile_pool(name="exp", bufs=3)
            epool = epool_cm.__enter__()
            rpool_cm = tc.tile_pool(name="rcp", bufs=2)
            rpool = rpool_cm.__enter__()
            rbpool_cm = tc.tile_pool(name="rbc", bufs=3)
            rbpool = rbpool_cm.__enter__()
            napool_cm = tc.tile_pool(name="tmp", bufs=3)
            napool = napool_cm.__enter__()

            XT = xpool.tile([128, 8, S], bf16, tag="XT")

            def load_w(param):
                # one [128, 8, 1024] tile, 2 descriptors (chains read all 8
                # sub-tiles anyway, so coarse descs cost no pipelining)
                t = wpool.tile([128, 8, 1024], bf16, tag="w", name="wtile")
                for j in range(2):
                    src_ap = bass.AP(
                        tensor=param.ap().tensor,
                        offset=j * 4 * 128 * 1024,
                        ap=[[1024, 128], [128 * 1024, 4], [1, 1024]],
                    )
                    nc.sync.dma_start(out=t[:, j * 4:(j + 1) * 4, :], in_=src_ap)
                return t

            def load_xt(half):
                for j in range(2):
                    src_ap = bass.AP(
                        tensor=xT_p.ap().tensor,
                        offset=j * 4 * 128 * 1024 + half * 512,
                        ap=[[1024, 128], [128 * 1024, 4], [1, 512]],
                    )
                    nc.sync.dma_start(
                        out=XT[:, j * 4:(j + 1) * 4, half * 512:(half + 1) * 512],
                        in_=src_ap)

            wq_t = load_w(wq_p)
            load_xt(0)
            wk_t = load_w(wk_p)
            load_xt(1)
            wv_t = None

            for tt in range(8):
                nc.vector.memset(VG_t[tt][:, :, 64:65], 1.0)

            # ---- emitters ----
            def emit_qproj():
                for ot in range(8):
                    ps = apool.tile([128, HALF], f32, tag="acc")
                    for it in range(8):
                        nc.tensor.matmul(
                            ps,
                            lhsT=wq_t[:, it, ot * 128:(ot + 1) * 128],
                            rhs=XT[:, it, 0:HALF],
                            start=(it == 0), stop=(it == 7),
                        )
                    nc.scalar.add(QT_t[ot], ps, BQ[:, ot:ot + 1])

            def emit_kproj(ot):
                psA = apool.tile([128, HALF], f32, tag="acc", name=f"kpsA{ot}")
                psB = apool.tile([128, HALF], f32, tag="acc", name=f"kpsB{ot}")
                for it in range(8):
                    for ncr, ps in ((0, psA), (1, psB)):
                        nc.tensor.matmul(
                            ps,
                            lhsT=wk_t[:, it, ot * 128:(ot + 1) * 128],
                            rhs=XT[:, it, ncr * 512:(ncr + 1) * 512],
                            start=(it == 0), stop=(it == 7),
                        )
                nc.scalar.copy(KT_t[ot][:, 0:512], psA)
                nc.scalar.copy(KT_t[ot][:, 512:1024], psB)

            def emit_vproj(ncr):
                for tt in range(8):
                    ps = apool.tile([128, HALF], f32, tag="acc")
                    for it in range(8):
                        nc.tensor.matmul(
                            ps,
                            lhsT=XT[:, it, tt * 128:(tt + 1) * 128],
                            rhs=wv_t[:, it, ncr * 512:(ncr + 1) * 512],
                            start=(it == 0), stop=(it == 7),
                        )
                    ps3 = ps.rearrange("p (h c) -> p h c", c=64)
                    nc.vector.tensor_tensor(
                        VG_t[tt][:, ncr * 8:(ncr + 1) * 8, 0:64], ps3,
                        BV[:, ncr * 8:(ncr + 1) * 8, :], ADD)

            def emit_scores(pr):
                EXPp = epool.tile([128, 2, 8, HALF], bf16, tag="exp", name=f"EXP{pr}")
                for kt in range(8):
                    sp = spool.tile([128, 2, HALF], f32, tag="sps")
                    for hh in range(2):
                        nc.tensor.matmul(
                            sp[:, hh, :],
                            lhsT=KT_t[pr][hh * 64:hh * 64 + 64, kt * 128:(kt + 1) * 128],
                            rhs=QT_t[pr][hh * 64:hh * 64 + 64, :],
                            start=True, stop=True,
                        )
                    nc.scalar.activation(
                        EXPp[:, :, kt, :], sp,
                        mybir.ActivationFunctionType.Exp,
                    )
                return EXPp

            def emit_pvc(pr, EXPp):
                # PV both heads, pair-batched reciprocal chain, CTX + ASUM
                cps = []
                dcp = rpool.tile([1, 2, HALF], f32, tag="dcp", name=f"dcp{pr}")
                for hh in range(2):
                    h = 2 * pr + hh
                    cp = cpool.tile([65, HALF], f32, tag="cps")
                    for kt in range(8):
                        nc.tensor.matmul(
                            cp,
                            lhsT=VG_t[kt][:, h, :],
                            rhs=EXPp[:, hh, kt, :],
                            start=(kt == 0), stop=(kt == 7),
                        )
                    nc.scalar.copy(dcp[:, hh, :], cp[64:65, :])
                    cps.append(cp)

                rec = rpool.tile([1, 2, HALF], f32, tag="rec", name=f"rec{pr}")
                nc.vector.reciprocal_approx_fast(
                    out=rec.rearrange("p a b -> p (a b)"),
                    in_=dcp.rearrange("p a b -> p (a b)"))
                rec_bf = rpool.tile([1, 2, HALF], bf16, tag="recbf", name=f"recbf{pr}")
                nc.vector.tensor_copy(
                    rec_bf.rearrange("p a b -> p (a b)"),
                    rec.rearrange("p a b -> p (a b)"))
                rbc = rbpool.tile([128, 2, HALF], bf16, tag="rbc", name=f"rbc{pr}")
                nc.gpsimd.partition_broadcast(
                    rbc.rearrange("p a b -> p (a b)"),
                    rec_bf.rearrange("p a b -> p (a b)")[0:1, :], channels=128)

                tmps = []
                for hh in range(2):
                    h = 2 * pr + hh
                    cp = cps[hh]
                    nc.vector.tensor_tensor(
                        CTX_t[pr][hh * 64:hh * 64 + 64, :], cp[0:64, :],
                        rbc[0:64, hh, :], MULT,
                    )
                    rbc_ap = rbc[:, hh, :]
                    rbc_w = bass.AP(tensor=rbc_ap.tensor, offset=rbc_ap.offset,
                                    ap=[rbc_ap.ap[0], [0, 8], rbc_ap.ap[1]])
                    tmpw = napool.tile([128, 8, HALF], bf16, tag="nrmattn")
                    nc.vector.tensor_tensor(tmpw, EXPp[:, hh, :, :], rbc_w, MULT)
                    tmps.append(tmpw)
                # pair sum in place, ship straight to DRAM (host adds the 8)
                nc.vector.tensor_tensor(tmps[0], tmps[1], tmps[0], ADD)
                nc.sync.dma_start(
                    out=bass.AP(
                        tensor=attn_p.ap().tensor, offset=pr * S * HALF,
                        ap=[[HALF, 128], [128 * HALF, 8], [1, HALF]],
                    ),
                    in_=tmps[0][:, :, :],
                )

            # ---- interleaved emission ----
            emit_qproj()
            wv_t = load_w(wv_p)
            emit_kproj(0)
            emit_kproj(1)
            exps = {}
            exps[0] = emit_scores(0)
            emit_vproj(0)
            exps[1] = emit_scores(1)
            emit_pvc(0, exps.pop(0))
            emit_kproj(2)
            emit_kproj(3)
            exps[2] = emit_scores(2)
            emit_pvc(1, exps.pop(1))
            emit_vproj(1)
            exps[3] = emit_scores(3)
            emit_pvc(2, exps.pop(2))
            emit_kproj(4)
            emit_kproj(5)
            exps[4] = emit_scores(4)
            emit_pvc(3, exps.pop(3))
            emit_kproj(6)
            emit_kproj(7)
            exps[5] = emit_scores(5)
            emit_pvc(4, exps.pop(4))
            exps[6] = emit_scores(6)
            emit_pvc(5, exps.pop(5))
            exps[7] = emit_scores(7)
            emit_pvc(6, exps.pop(6))
            emit_pvc(7, exps.pop(7))

            for cm in (napool_cm, rbpool_cm, rpool_cm, epool_cm, cpool_cm,
                       spool_cm, apool_cm, wpool_cm, xpool_cm):
                cm.__exit__(None, None, None)

            # wo + XR load (scoped after attention pools close)
            wopool_cm = tc.tile_pool(name="wo2", bufs=1)
            wopool = wopool_cm.__enter__()
            wo_t = wopool.tile([128, 8, 1024], bf16, tag="wo", name="wo_t")
            for j in range(2):
                src_ap = bass.AP(
                    tensor=wo_p.ap().tensor,
                    offset=j * 4 * 128 * 1024,
                    ap=[[1024, 128], [128 * 1024, 4], [1, 1024]],
                )
                nc.sync.dma_start(out=wo_t[:, j * 4:(j + 1) * 4, :], in_=src_ap)
            xrpool_cm = tc.tile_pool(name="xr", bufs=1)
            xrpool = xrpool_cm.__enter__()
            XR = xrpool.tile([128, 4, 1024], bf16, tag="XR")
            nc.sync.dma_start(
                out=XR,
                in_=bass.AP(tensor=xr_p.ap().tensor, offset=0,
                            ap=[[1024, 128], [128 * 1024, 4], [1, 1024]]))

            # ======== output projection + residual + LayerNorm ========
            with (
                tc.tile_pool(name="ln", bufs=2) as lpool,
                tc.tile_pool(name="ops", bufs=2, space="PSUM") as opool,
            ):

                for tt in range(4):
                    hh_t = lpool.tile([128, 1024], f32, tag="hh")
                    for ncr in range(2):
                        ps = opool.tile([128, HALF], f32, tag="ops")
                        for it in range(8):
                            nc.tensor.matmul(
                                ps,
                                lhsT=CTX_t[it][:, tt * 128:(tt + 1) * 128],
                                rhs=wo_t[:, it, ncr * 512:(ncr + 1) * 512],
                                start=(it == 0), stop=(it == 7),
                            )
                        nc.vector.tensor_tensor(
                            hh_t[:, ncr * 512:(ncr + 1) * 512], ps,
                            XR[:, tt, ncr * 512:(ncr + 1) * 512], ADD)

                    st = lpool.tile([128, 2, 6], f32, tag="st")
                    for g2 in range(2):
                        nc.vector.bn_stats(st[:, g2, :], hh_t[:, g2 * 512:(g2 + 1) * 512])
                    mv = lpool.tile([128, 2], f32, tag="mv")
                    nc.vector.bn_aggr(mv, st)
                    nmu = lpool.tile([128, 1], f32, tag="nmu")
                    nc.vector.tensor_scalar_mul(nmu, mv[:, 0:1], -1.0)
                    sd = lpool.tile([128, 1], f32, tag="sd")
                    nc.scalar.activation(
                        sd, mv[:, 1:2], mybir.ActivationFunctionType.Sqrt,
                        bias=EPS[:, 0:1], scale=1.0,
                    )
                    rstd = lpool.tile([128, 1], f32, tag="rstd")
                    nc.vector.reciprocal(rstd, sd)

                    t1 = lpool.tile([128, 1024], bf16 if ln_trivial else f32, tag="t1")
                    nc.vector.tensor_scalar(t1, hh_t, nmu[:, 0:1], rstd[:, 0:1], ADD, MULT)
                    if not ln_trivial:
                        t2 = lpool.tile([128, 1024], f32, tag="t2")
                        nc.vector.tensor_tensor(t2, t1, GLN, MULT)
                        nrm = lpool.tile([128, 1024], bf16, tag="nrm")
                        nc.vector.tensor_tensor(nrm, t2, BLN, ADD)
                        t1 = nrm
                    nc.sync.dma_start(out=normed_p[tt * 128:(tt + 1) * 128, :], in_=t1)

            xrpool_cm.__exit__(None, None, None)
            wopool_cm.__exit__(None, None, None)

    nc.finalize()
    return nc


def _get_nc(ln_trivial=True):
    key = f"nc_{ln_trivial}"
    if key not in _cache:
        _cache[key] = _build_nc(ln_trivial=ln_trivial)
    return _cache[key]


def _prep_in_maps(inputs):
    x = np.asarray(inputs["x"], dtype=np.float32)
    w_eff = {}
    for p in ("q", "k", "v", "o"):
        w = np.asarray(inputs[f"{p}_w"], dtype=np.float32)
        A = np.asarray(inputs[f"{p}_A"], dtype=np.float32)
        Bm = np.asarray(inputs[f"{p}_B"], dtype=np.float32)
        w_eff[p] = w.T + 2.0 * (A @ Bm)          # [in, out]
    import ml_dtypes
    bf = ml_dtypes.bfloat16
    wq = (w_eff["q"] / 8.0).astype(bf)
    wk = w_eff["k"].astype(bf)
    wv = w_eff["v"].astype(bf)
    wo = w_eff["o"].astype(bf)
    bqa = (np.asarray(inputs["q_b"], np.float32) / 8.0).reshape(8, 128).T.copy()
    bv = np.ascontiguousarray(inputs["v_b"], dtype=np.float32)
    ob = np.asarray(inputs["o_b"], np.float32)
    lng = np.ascontiguousarray(inputs["ln_g"], dtype=np.float32)
    lnb = np.ascontiguousarray(inputs["ln_b"], dtype=np.float32)

    in_maps = []
    for c in range(N_CORES):
        b, qh = c // 2, c % 2
        xb = x[b]                                  # [S, D]
        xT = np.ascontiguousarray(xb.T)            # [D, S]
        if qh == 1:
            xT = np.concatenate([xT[:, HALF:], xT[:, :HALF]], axis=1)
        xT = xT.astype(bf)
        xr = np.ascontiguousarray(xb[qh * HALF:(qh + 1) * HALF, :] + ob[None, :]).astype(bf)
        in_maps.append({
            "xT": xT, "xr": xr,
            "wq": wq, "wk": wk, "wv": wv, "wo": wo,
            "bqa": bqa, "bv": bv,
            "lng": lng, "lnb": lnb,
        })
    return in_maps


def run_on_device(inputs, trace=False, tmpdir=None):
    from concourse.bass_utils import run_bass_kernel_spmd

    ln_trivial = bool(
        np.all(np.asarray(inputs["ln_g"]) == 1.0)
        and np.all(np.asarray(inputs["ln_b"]) == 0.0))
    nc = _get_nc(ln_trivial=ln_trivial)
    in_maps = _prep_in_maps(inputs)
    res = run_bass_kernel_spmd(
        nc, in_maps, core_ids=list(range(N_CORES)), trace=trace, tmpdir=tmpdir,
    )

    normed = np.zeros((B, S, D), dtype=np.float32)
    attn_mean = np.zeros((B, S, S), dtype=np.float32)
    for c in range(N_CORES):
        b, qh = c // 2, c % 2
        normed[b, qh * HALF:(qh + 1) * HALF, :] = np.asarray(
            res.results[c]["normed"], dtype=np.float32)
    for b in range(B):
        halves = []
        for qh in range(2):
            r = res.results[2 * b + qh]
            A = np.asarray(r["attn_out"], dtype=np.float32).sum(axis=0) * (1.0 / 16.0)
            if qh == 1:
                A = np.concatenate([A[HALF:], A[:HALF]], axis=0)  # undo k-perm
            halves.append(A)                       # [S(k), HALF(q)]
        attn_mean[b] = np.concatenate(halves, axis=1).T
    return (normed, attn_mean), res


def kernel(**inputs):
    (normed, attn_mean), _ = run_on_device(inputs, trace=False)
    return normed, attn_mean


# revision 5
# speedup vs baseline: 1.1749x; 1.0024x over previous
"""Trainium2 Bass kernel for EnhancedMultiHeadAttention (LoRA MHA + residual + LayerNorm).

Contract: kernel(**inputs) takes the FULL unsharded inputs (as produced by
setup_inputs()) and returns the full outputs (normed, attn.mean(axis=1)).

Sharding: 8 cores = 4 batches x 2 query-halves. Each core computes K/V for the
whole sequence of its batch (duplicated across the pair) and attention +
output-projection + LayerNorm for its 512 query rows.

Key optimizations (231us -> ~207us):
  - K bias dropped entirely: softmax over k is invariant to the per-q constant
    q.k_b adds to scores, so neither output depends on it.
  - Interleaved emission: engine queues execute in emission order, so
    scores/exp/PV/normalize chunks are emitted BETWEEN projection chunks;
    the PE stays dense (no HAM clock-throttle oscillation mid-kernel).
  - Coarse DMA descriptors (2 per weight / x^T half): serialized descriptor
    issue on the Sync engine was delaying the K projection by ~15us.
  - Softmax reciprocal chain pair-batched; the 1/denom partition-broadcast
    runs on the otherwise-idle GpSimd engine instead of PE+ScalarE.
  - Attention-mean kept as 8 pair-level partial sums shipped straight to
    DRAM (host sums them); the last pairs' partials are DEFERRED past the
    O-projection emission so the output path never waits behind them in
    the vector-engine queue.
  - wo unscaled; the 1/16 head-mean factor folded into the host gather;
    normed output and the x-residual carried bf16.
"""



import sys
import numpy as np

_REPO = "/opt/trn_rl_repo"
if _REPO not in sys.path:
    sys.path.insert(0, _REPO)

D = 1024       # d_model
S = 1024       # sequence length
B = 4          # batch
H = 16         # heads
DK = 64        # head dim
HALF = 512     # query rows per core
N_CORES = 8
LN_EPS = 1e-5

_cache = {}
GPSIMD_ADDS = 0   # number of ASUM ADD ops offloaded to GpSimd (0-14)


def _build_nc(ln_trivial=True):
    import concourse.bacc as bacc
    import concourse.mybir as mybir
    import concourse.tile as tile
    import concourse.bass as bass

    f32 = mybir.dt.float32
    bf16 = mybir.dt.bfloat16
    ADD = mybir.AluOpType.add
    MULT = mybir.AluOpType.mult

    nc = bacc.Bacc(None, target_bir_lowering=False)

    # ---- DRAM parameters (per-core views, SPMD-identical program) ----
    xT_p = nc.declare_dram_parameter("xT", [D, S], bf16, isOutput=False)      # x[b].T, query-half-first token order
    xr_p = nc.declare_dram_parameter("xr", [HALF, D], bf16, isOutput=False)    # x rows of our queries + o_bias
    wq_p = nc.declare_dram_parameter("wq", [D, D], bf16, isOutput=False)      # (q_w.T + 2 qA@qB)/8
    wk_p = nc.declare_dram_parameter("wk", [D, D], bf16, isOutput=False)      # k_w.T + 2 kA@kB
    wv_p = nc.declare_dram_parameter("wv", [D, D], bf16, isOutput=False)      # v_w.T + 2 vA@vB
    wo_p = nc.declare_dram_parameter("wo", [D, D], bf16, isOutput=False)      # o_w.T + 2 oA@oB
    bqa_p = nc.declare_dram_parameter("bqa", [128, 8], f32, isOutput=False)   # (q_b/8) arranged [p, ot]
    bv_p = nc.declare_dram_parameter("bv", [D], f32, isOutput=False)          # v_b
    lng_p = nc.declare_dram_parameter("lng", [D], f32, isOutput=False)        # ln gamma
    lnb_p = nc.declare_dram_parameter("lnb", [D], f32, isOutput=False)        # ln beta
    normed_p = nc.declare_dram_parameter("normed", [HALF, D], bf16, isOutput=True)
    attn_p = nc.declare_dram_parameter("attn_out", [8, S, HALF], bf16, isOutput=True)  # per-pair attn sums, [pr, k, q]

    def bcast_ap(handle, dims):
        # broadcast a [D] dram vector across 128 partitions; dims shapes the free side
        ap = handle.ap()
        return bass.AP(tensor=ap.tensor, offset=ap.offset, ap=[[0, 128]] + dims)

    with tile.TileContext(nc) as tc:
        with (
            tc.tile_pool(name="consts", bufs=1) as consts,
            tc.tile_pool(name="res", bufs=1) as res,
        ):
            BQ = consts.tile([128, 8], f32, tag="bq")
            nc.sync.dma_start(out=BQ, in_=bqa_p[:, :])
            BV = consts.tile([128, 16, 64], f32, tag="bv")
            nc.sync.dma_start(out=BV, in_=bcast_ap(bv_p, [[64, 16], [1, 64]]))
            if not ln_trivial:
                GLN = consts.tile([128, 1024], f32, tag="gln")
                nc.sync.dma_start(out=GLN, in_=bcast_ap(lng_p, [[1, 1024]]))
                BLN = consts.tile([128, 1024], f32, tag="bln")
                nc.sync.dma_start(out=BLN, in_=bcast_ap(lnb_p, [[1, 1024]]))
            EPS = consts.tile([128, 1], f32, tag="eps")
            nc.vector.memset(EPS, LN_EPS)

            # ---------------- persistent activations ----------------
            KT_t = [res.tile([128, S], bf16, tag=f"KT{ot}", name=f"KT{ot}") for ot in range(8)]
            QT_t = [res.tile([128, HALF], bf16, tag=f"QT{ot}", name=f"QT{ot}") for ot in range(8)]
            VG_t = [res.tile([128, 16, 65], bf16, tag=f"VG{tt}", name=f"VG{tt}") for tt in range(8)]
            CTX_t = [res.tile([128, HALF], bf16, tag=f"CTX{ot}", name=f"CTX{ot}") for ot in range(8)]

            # ======== projections + attention, interleaved emission ========
            # Engine queues execute in emission order, so attention chunks are
            # emitted BETWEEN projection chunks: exp starts ~22us in, PE stays
            # dense (HAM warm), and the DVE chains trail the exp stream.
            epool_cm = tc.tile_pool(name="exp", bufs=3)
            epool = epool_cm.__enter__()
            rpool_cm = tc.tile_pool(name="rcp", bufs=2)
            rpool = rpool_cm.__enter__()
            rbpool_cm = tc.tile_pool(name="rbc", bufs=3)
            rbpool = rbpool_cm.__enter__()
            napool_cm = tc.tile_pool(name="tmp", bufs=3)
            napool = napool_cm.__enter__()
            xpool_cm = tc.tile_pool(name="xt", bufs=1)
            xpool = xpool_cm.__enter__()
            wpool_cm = tc.tile_pool(name="wts", bufs=2)
            wpool = wpool_cm.__enter__()
            apool_cm = tc.tile_pool(name="acc", bufs=2, space="PSUM")
            apool = apool_cm.__enter__()
            spool_cm = tc.tile_pool(name="sps", bufs=2, space="PSUM")
            spool = spool_cm.__enter__()
            cpool_cm = tc.tile_pool(name="cps", bufs=2, space="PSUM")
            cpool = cpool_cm.__enter__()

            XT = xpool.tile([128, 8, S], bf16, tag="XT")

            def load_w(param):
                # one [128, 8, 1024] tile, 2 descriptors (chains read all 8
                # sub-tiles anyway, so coarse descs cost no pipelining)
                t = wpool.tile([128, 8, 1024], bf16, tag="w", name="wtile")
                for j in range(2):
                    src_ap = bass.AP(
                        tensor=param.ap().tensor,
                        offset=j * 4 * 128 * 1024,
                        ap=[[1024, 128], [128 * 1024, 4], [1, 1024]],
                    )
                    nc.sync.dma_start(out=t[:, j * 4:(j + 1) * 4, :], in_=src_ap)
                return t

            def load_xt(half):
                for j in range(2):
                    src_ap = bass.AP(
                        tensor=xT_p.ap().tensor,
                        offset=j * 4 * 128 * 1024 + half * 512,
                        ap=[[1024, 128], [128 * 1024, 4], [1, 512]],
                    )
                    nc.sync.dma_start(
                        out=XT[:, j * 4:(j + 1) * 4, half * 512:(half + 1) * 512],
                        in_=src_ap)

            wq_t = load_w(wq_p)
            load_xt(0)
            wk_t = load_w(wk_p)
            load_xt(1)
            wv_t = None

            for tt in range(8):
                nc.vector.memset(VG_t[tt][:, :, 64:65], 1.0)

            # ---- emitters ----
            def emit_qproj():
                for ot in range(8):
                    ps = apool.tile([128, HALF], f32, tag="acc")
                    for it in range(8):
                        nc.tensor.matmul(
                            ps,
                            lhsT=wq_t[:, it, ot * 128:(ot + 1) * 128],
                            rhs=XT[:, it, 0:HALF],
                            start=(it == 0), stop=(it == 7),
                        )
                    nc.scalar.add(QT_t[ot], ps, BQ[:, ot:ot + 1])

            def emit_kproj(ot):
                psA = apool.tile([128, HALF], f32, tag="acc", name=f"kpsA{ot}")
                psB = apool.tile([128, HALF], f32, tag="acc", name=f"kpsB{ot}")
                for it in range(8):
                    for ncr, ps in ((0, psA), (1, psB)):
                        nc.tensor.matmul(
                            ps,
                            lhsT=wk_t[:, it, ot * 128:(ot + 1) * 128],
                            rhs=XT[:, it, ncr * 512:(ncr + 1) * 512],
                            start=(it == 0), stop=(it == 7),
                        )
                nc.scalar.copy(KT_t[ot][:, 0:512], psA)
                nc.scalar.copy(KT_t[ot][:, 512:1024], psB)

            def emit_vproj(ncr):
                for tt in range(8):
                    ps = apool.tile([128, HALF], f32, tag="acc")
                    for it in range(8):
                        nc.tensor.matmul(
                            ps,
                            lhsT=XT[:, it, tt * 128:(tt + 1) * 128],
                            rhs=wv_t[:, it, ncr * 512:(ncr + 1) * 512],
                            start=(it == 0), stop=(it == 7),
                        )
                    ps3 = ps.rearrange("p (h c) -> p h c", c=64)
                    nc.vector.tensor_tensor(
                        VG_t[tt][:, ncr * 8:(ncr + 1) * 8, 0:64], ps3,
                        BV[:, ncr * 8:(ncr + 1) * 8, :], ADD)

            def emit_scores(pr):
                EXPp = epool.tile([128, 2, 8, HALF], bf16, tag="exp", name=f"EXP{pr}")
                for kt in range(8):
                    sp = spool.tile([128, 2, HALF], f32, tag="sps")
                    for hh in range(2):
                        nc.tensor.matmul(
                            sp[:, hh, :],
                            lhsT=KT_t[pr][hh * 64:hh * 64 + 64, kt * 128:(kt + 1) * 128],
                            rhs=QT_t[pr][hh * 64:hh * 64 + 64, :],
                            start=True, stop=True,
                        )
                    nc.scalar.activation(
                        EXPp[:, :, kt, :], sp,
                        mybir.ActivationFunctionType.Exp,
                    )
                return EXPp

            def emit_pvc(pr, EXPp, defer=False):
                # PV both heads, pair-batched reciprocal chain, CTX + ASUM
                cps = []
                dcp = rpool.tile([1, 2, HALF], f32, tag="dcp", name=f"dcp{pr}")
                for hh in range(2):
                    h = 2 * pr + hh
                    cp = cpool.tile([65, HALF], f32, tag="cps")
                    for kt in range(8):
                        nc.tensor.matmul(
                            cp,
                            lhsT=VG_t[kt][:, h, :],
                            rhs=EXPp[:, hh, kt, :],
                            start=(kt == 0), stop=(kt == 7),
                        )
                    nc.scalar.copy(dcp[:, hh, :], cp[64:65, :])
                    cps.append(cp)

                rec = rpool.tile([1, 2, HALF], f32, tag="rec", name=f"rec{pr}")
                nc.vector.reciprocal_approx_fast(
                    out=rec.rearrange("p a b -> p (a b)"),
                    in_=dcp.rearrange("p a b -> p (a b)"))
                rec_bf = rpool.tile([1, 2, HALF], bf16, tag="recbf", name=f"recbf{pr}")
                nc.vector.tensor_copy(
                    rec_bf.rearrange("p a b -> p (a b)"),
                    rec.rearrange("p a b -> p (a b)"))
                rbc = rbpool.tile([128, 2, HALF], bf16, tag="rbc", name=f"rbc{pr}")
                nc.gpsimd.partition_broadcast(
                    rbc.rearrange("p a b -> p (a b)"),
                    rec_bf.rearrange("p a b -> p (a b)")[0:1, :], channels=128)

                for hh in range(2):
                    cp = cps[hh]
                    nc.vector.tensor_tensor(
                        CTX_t[pr][hh * 64:hh * 64 + 64, :], cp[0:64, :],
                        rbc[0:64, hh, :], MULT,
                    )
                if defer:
                    return (EXPp, rbc)
                emit_asum(pr, EXPp, rbc)
                return None

            def emit_asum(pr, EXPp, rbc):
                # attn-mean partial for the pair (gates only the attn output,
                # not normed — deferrable past the O projection)
                tmps = []
                for hh in range(2):
                    rbc_ap = rbc[:, hh, :]
                    rbc_w = bass.AP(tensor=rbc_ap.tensor, offset=rbc_ap.offset,
                                    ap=[rbc_ap.ap[0], [0, 8], rbc_ap.ap[1]])
                    tmpw = napool.tile([128, 8, HALF], bf16, tag="nrmattn")
                    nc.vector.tensor_tensor(tmpw, EXPp[:, hh, :, :], rbc_w, MULT)
                    tmps.append(tmpw)
                nc.vector.tensor_tensor(tmps[0], tmps[1], tmps[0], ADD)
                nc.sync.dma_start(
                    out=bass.AP(
                        tensor=attn_p.ap().tensor, offset=pr * S * HALF,
                        ap=[[HALF, 128], [128 * HALF, 8], [1, HALF]],
                    ),
                    in_=tmps[0][:, :, :],
                )

            # ---- interleaved emission ----
            emit_qproj()
            wv_t = load_w(wv_p)
            emit_kproj(0)
            emit_kproj(1)
            exps = {}
            exps[0] = emit_scores(0)
            emit_vproj(0)
            exps[1] = emit_scores(1)
            emit_pvc(0, exps.pop(0))
            emit_kproj(2)
            emit_kproj(3)
            exps[2] = emit_scores(2)
            emit_pvc(1, exps.pop(1))
            emit_vproj(1)
            exps[3] = emit_scores(3)
            emit_pvc(2, exps.pop(2))
            emit_kproj(4)
            emit_kproj(5)
            exps[4] = emit_scores(4)
            emit_pvc(3, exps.pop(3))
            emit_kproj(6)
            emit_kproj(7)
            exps[5] = emit_scores(5)
            emit_pvc(4, exps.pop(4))
            exps[6] = emit_scores(6)
            emit_pvc(5, exps.pop(5))
            exps[7] = emit_scores(7)
            d6 = emit_pvc(6, exps.pop(6), defer=True)
            d7 = emit_pvc(7, exps.pop(7), defer=True)

            for cm in (cpool_cm, spool_cm, apool_cm, wpool_cm, xpool_cm):
                cm.__exit__(None, None, None)

            # wo + XR load (scoped after attention pools close)
            wopool_cm = tc.tile_pool(name="wo2", bufs=1)
            wopool = wopool_cm.__enter__()
            wo_t = wopool.tile([128, 8, 1024], bf16, tag="wo", name="wo_t")
            for j in range(2):
                src_ap = bass.AP(
                    tensor=wo_p.ap().tensor,
                    offset=j * 4 * 128 * 1024,
                    ap=[[1024, 128], [128 * 1024, 4], [1, 1024]],
                )
                nc.sync.dma_start(out=wo_t[:, j * 4:(j + 1) * 4, :], in_=src_ap)
            xrpool_cm = tc.tile_pool(name="xr", bufs=1)
            xrpool = xrpool_cm.__enter__()
            XR = xrpool.tile([128, 4, 1024], bf16, tag="XR")
            nc.sync.dma_start(
                out=XR,
                in_=bass.AP(tensor=xr_p.ap().tensor, offset=0,
                            ap=[[1024, 128], [128 * 1024, 4], [1, 1024]]))

            # ======== output projection + residual + LayerNorm ========
            with (
                tc.tile_pool(name="ln", bufs=2) as lpool,
                tc.tile_pool(name="ops", bufs=2, space="PSUM") as opool,
            ):

                for tt in range(4):
                    hh_t = lpool.tile([128, 1024], f32, tag="hh")
                    for ncr in range(2):
                        ps = opool.tile([128, HALF], f32, tag="ops")
                        for it in range(8):
                            nc.tensor.matmul(
                                ps,
                                lhsT=CTX_t[it][:, tt * 128:(tt + 1) * 128],
                                rhs=wo_t[:, it, ncr * 512:(ncr + 1) * 512],
                                start=(it == 0), stop=(it == 7),
                            )
                        nc.vector.tensor_tensor(
                            hh_t[:, ncr * 512:(ncr + 1) * 512], ps,
                            XR[:, tt, ncr * 512:(ncr + 1) * 512], ADD)

                    st = lpool.tile([128, 2, 6], f32, tag="st")
                    for g2 in range(2):
                        nc.vector.bn_stats(st[:, g2, :], hh_t[:, g2 * 512:(g2 + 1) * 512])
                    mv = lpool.tile([128, 2], f32, tag="mv")
                    nc.vector.bn_aggr(mv, st)
                    nmu = lpool.tile([128, 1], f32, tag="nmu")
                    nc.vector.tensor_scalar_mul(nmu, mv[:, 0:1], -1.0)
                    sd = lpool.tile([128, 1], f32, tag="sd")
                    nc.scalar.activation(
                        sd, mv[:, 1:2], mybir.ActivationFunctionType.Sqrt,
                        bias=EPS[:, 0:1], scale=1.0,
                    )
                    rstd = lpool.tile([128, 1], f32, tag="rstd")
                    nc.vector.reciprocal(rstd, sd)

                    t1 = lpool.tile([128, 1024], bf16 if ln_trivial else f32, tag="t1")
                    nc.vector.tensor_scalar(t1, hh_t, nmu[:, 0:1], rstd[:, 0:1], ADD, MULT)
                    if not ln_trivial:
                        t2 = lpool.tile([128, 1024], f32, tag="t2")
                        nc.vector.tensor_tensor(t2, t1, GLN, MULT)
                        nrm = lpool.tile([128, 1024], bf16, tag="nrm")
                        nc.vector.tensor_tensor(nrm, t2, BLN, ADD)
                        t1 = nrm
                    nc.sync.dma_start(out=normed_p[tt * 128:(tt + 1) * 128, :], in_=t1)

            # deferred attn-mean partials for the last two pairs (they gate
            # only the attn output, so they run behind the O projection)
            emit_asum(6, *d6)
            emit_asum(7, *d7)

            xrpool_cm.__exit__(None, None, None)
            wopool_cm.__exit__(None, None, None)
            for cm in (napool_cm, rbpool_cm, rpool_cm, epool_cm):
                cm.__exit__(None, None, None)

    nc.finalize()
    return nc


def _get_nc(ln_trivial=True):
    key = f"nc_{ln_trivial}"
    if key not in _cache:
        _cache[key] = _build_nc(ln_trivial=ln_trivial)
    return _cache[key]


def _prep_in_maps(inputs):
    x = np.asarray(inputs["x"], dtype=np.float32)
    w_eff = {}
    for p in ("q", "k", "v", "o"):
        w = np.asarray(inputs[f"{p}_w"], dtype=np.float32)
        A = np.asarray(inputs[f"{p}_A"], dtype=np.float32)
        Bm = np.asarray(inputs[f"{p}_B"], dtype=np.float32)
        w_eff[p] = w.T + 2.0 * (A @ Bm)          # [in, out]
    import ml_dtypes
    bf = ml_dtypes.bfloat16
    wq = (w_eff["q"] / 8.0).astype(bf)
    wk = w_eff["k"].astype(bf)
    wv = w_eff["v"].astype(bf)
    wo = w_eff["o"].astype(bf)
    bqa = (np.asarray(inputs["q_b"], np.float32) / 8.0).reshape(8, 128).T.copy()
    bv = np.ascontiguousarray(inputs["v_b"], dtype=np.float32)
    ob = np.asarray(inputs["o_b"], np.float32)
    lng = np.ascontiguousarray(inputs["ln_g"], dtype=np.float32)
    lnb = np.ascontiguousarray(inputs["ln_b"], dtype=np.float32)

    in_maps = []
    for c in range(N_CORES):
        b, qh = c // 2, c % 2
        xb = x[b]                                  # [S, D]
        xT = np.ascontiguousarray(xb.T)            # [D, S]
        if qh == 1:
            xT = np.concatenate([xT[:, HALF:], xT[:, :HALF]], axis=1)
        xT = xT.astype(bf)
        xr = np.ascontiguousarray(xb[qh * HALF:(qh + 1) * HALF, :] + ob[None, :]).astype(bf)
        in_maps.append({
            "xT": xT, "xr": xr,
            "wq": wq, "wk": wk, "wv": wv, "wo": wo,
            "bqa": bqa, "bv": bv,
            "lng": lng, "lnb": lnb,
        })
    return in_maps


def run_on_device(inputs, trace=False, tmpdir=None):
    from concourse.bass_utils import run_bass_kernel_spmd

    ln_trivial = bool(
        np.all(np.asarray(inputs["ln_g"]) == 1.0)
        and np.all(np.asarray(inputs["ln_b"]) == 0.0))
    nc = _get_nc(ln_trivial=ln_trivial)
    in_maps = _prep_in_maps(inputs)
    res = run_bass_kernel_spmd(
        nc, in_maps, core_ids=list(range(N_CORES)), trace=trace, tmpdir=tmpdir,
    )

    normed = np.zeros((B, S, D), dtype=np.float32)
    attn_mean = np.zeros((B, S, S), dtype=np.float32)
    for c in range(N_CORES):
        b, qh = c // 2, c % 2
        normed[b, qh * HALF:(qh + 1) * HALF, :] = np.asarray(
            res.results[c]["normed"], dtype=np.float32)
    for b in range(B):
        halves = []
        for qh in range(2):
            r = res.results[2 * b + qh]
            A = np.asarray(r["attn_out"], dtype=np.float32).sum(axis=0) * (1.0 / 16.0)
            if qh == 1:
                A = np.concatenate([A[HALF:], A[:HALF]], axis=0)  # undo k-perm
            halves.append(A)                       # [S(k), HALF(q)]
        attn_mean[b] = np.concatenate(halves, axis=1).T
    return (normed, attn_mean), res


def kernel(**inputs):
    (normed, attn_mean), _ = run_on_device(inputs, trace=False)
    return normed, attn_mean
